# revision 17
# baseline (speedup 1.0000x reference)
"""3-layer GCN encoder (GCNConv + LayerNorm + ReLU) on 8 TRN2 NeuronCores.

Strategy (dst-partitioned graph parallel, v2):
  - Nodes are packed into 128-row tiles by a host-side bin-packing that
    equalizes per-(tile,bank) gather-group sizes across cores (bimodal tile
    in-degree targets), cutting shared-schedule chunk padding from ~32% to
    ~14%. Tiles are striped across the 8 cores.
  - Self-loop messages never enter the gather stream: each dst tile's PSUM
    accumulator is initialized with hhat_tile via an identity matmul
    (conv = dinv_dst * (sum_gathered hhat_src + hhat_dst) + b).
  - Per layer l: each core computes h_c = x_c @ W_l (bf16), scales rows by
    dinv (deg^-1/2), keeps the bf16 result in SBUF (hhat_sb) and AllGathers
    it so every core holds the full scaled table hhat [100352, 128] bf16 in
    DRAM.
  - Edge phase: edges grouped by (dst_tile, src_bank); each 128-edge chunk
    is gathered from hfull via dma_gather (int16 indices, 4 banks of 25088
    rows) and scatter-added into the dst tile's PSUM via a one-hot matmul.
  - Per dst tile: conv = PSUM * dinv_dst + b, LayerNorm (+ReLU), PE
    transpose back into the feature-major bf16 x_cT buffer, and the NEXT
    layer's x@W matmul runs immediately (interleaved into the edge loop,
    emit-shifted by one tile so the LN chain never stalls the PE stream).

kernel(**inputs) takes the FULL inputs and returns the FULL [100000, 128]
float32 output.
"""
import os
import sys

sys.path.insert(0, "/opt/trn_rl_repo")

import numpy as np
import ml_dtypes

N = 100000
D = 128
NCORES = 8
P = 128
TILES = 98           # tiles per core
NPAD = TILES * P     # 12544 padded nodes per core
NG = NCORES * NPAD   # 100352 global padded rows
NBANK = 4
QB = [0, 25, 50, 74, 98]          # tile-quarter boundaries (per core)
QROWS = [3200, 3200, 3072, 3072]  # rows per quarter per core
BANKQ = [8 * r for r in QROWS]    # bank sizes (< 32767, int16-addressable)
EPS = 1e-5
NLO = 60             # low-degree tiles per core (cells ~450 -> 4 chunks)
T_LO = 1720.0

GATHER_GROUP = int(os.environ.get("GCN_G", "32"))   # chunks per dma_gather
SINGLE_PACKET = bool(int(os.environ.get("GCN_SP", "0")))
S_BATCH = int(os.environ.get("GCN_SB", "16"))       # chunks per is_equal
GBUFS = int(os.environ.get("GCN_GBUFS", "8"))
DMA_SCRATCH = int(os.environ.get("GCN_SCRATCH", "32768"))  # desc-ring carveout
NLAYERS = int(os.environ.get("GCN_LAYERS", "3"))


def _pack_tiles(degp):
    """Assign nodes to (core, slot-in-core, pos) equalizing per-tile in-degree
    sums with bimodal targets, so per-(core,tile,bank) gather groups quantize
    tightly to 128-row chunks under the shared (max-over-cores) schedule."""
    import heapq

    ntiles = NCORES * TILES
    nlo = NCORES * NLO
    tot = float(degp.sum())
    t_hi = (tot / NCORES - NLO * T_LO) / (TILES - NLO)
    targets = np.empty(ntiles)
    targets[:nlo] = T_LO
    targets[nlo:] = t_hi

    order = np.argsort(-degp, kind="stable")
    heap = [(-targets[t], t) for t in range(ntiles)]
    heapq.heapify(heap)
    slots_left = np.full(ntiles, P, np.int32)
    cur = np.zeros(ntiles)
    tile_of = np.empty(N, np.int32)
    for v in order:
        while True:
            negdef, t = heapq.heappop(heap)
            if slots_left[t] > 0:
                break
        tile_of[v] = t
        slots_left[t] -= 1
        cur[t] += degp[v]
        if slots_left[t] > 0:
            heapq.heappush(heap, (-(targets[t] - cur[t]), t))

    gt = np.arange(ntiles)
    core_of_tile = np.where(gt < nlo, gt % NCORES, (gt - nlo) % NCORES)
    slot_of_tile = np.where(gt < nlo, gt // NCORES, NLO + (gt - nlo) // NCORES)

    pos_of = np.empty(N, np.int64)
    cnt_in_tile = np.zeros(ntiles, np.int64)
    for v in np.argsort(tile_of, kind="stable"):
        t = tile_of[v]
        pos_of[v] = cnt_in_tile[t]
        cnt_in_tile[t] += 1

    core_of = core_of_tile[tile_of].astype(np.int64)
    slot_of = slot_of_tile[tile_of].astype(np.int64)
    return core_of, slot_of, pos_of


def _preprocess(x, edge_index):
    """Host-side graph preprocessing. Returns per-core input arrays and the
    shared chunk schedule. Self-loops are excluded from the gather stream."""
    ei = np.asarray(edge_index)
    src = np.asarray(ei[0], np.int64)   # non-self edges only
    dst = np.asarray(ei[1], np.int64)
    M = src.shape[0]

    deg = (np.bincount(dst, minlength=N) + 1).astype(np.float32)  # incl self
    dinv = 1.0 / np.sqrt(deg)

    degp = np.bincount(dst, minlength=N).astype(np.int64)
    core_of, slot_of, pos_of = _pack_tiles(degp)
    sidx_of = slot_of * P + pos_of          # row within the core's slice
    # bank = tile-quarter of the source node; row within bank is
    # core-major over that quarter's rows (matching the quarter AllGather)
    q_of_slot = np.zeros(TILES, np.int64)
    for q in range(NBANK):
        q_of_slot[QB[q]:QB[q + 1]] = q
    qb_rows = np.array([QB[q] * P for q in range(NBANK)], np.int64)
    qrows = np.array(QROWS, np.int64)

    node_q = q_of_slot[slot_of]
    node_srel = core_of * qrows[node_q] + (sidx_of - qb_rows[node_q])

    core = core_of[dst]
    t = slot_of[dst]
    drel = pos_of[dst]
    b = node_q[src]
    srel = node_srel[src]

    key = (core * TILES + t) * NBANK + b
    order = np.argsort(key, kind="stable")
    key_s = key[order]
    core_s = core[order]
    srel_s = srel[order]
    drel_s = drel[order]

    cnt = np.bincount(key, minlength=NCORES * TILES * NBANK).reshape(
        NCORES, TILES, NBANK
    )
    K = np.ceil(cnt.max(axis=0) / P).astype(np.int64)  # [TILES, NBANK] shared
    Ltb = (K * P).reshape(-1)                          # padded group lengths
    off2 = np.concatenate([[0], np.cumsum(Ltb)[:-1]])  # group offsets (flat t,b)
    TOT = int(Ltb.sum())                               # padded edges per core
    TOTCH = TOT // P

    # rank of each edge within its (core, t, b) group
    first = np.searchsorted(key_s, key_s, side="left")
    rank = np.arange(M) - first
    pos = off2[(key_s % (TILES * NBANK))] + rank

    srcrel_pad = np.zeros((NCORES, TOT), np.int16)
    dstrel_pad = np.full((NCORES, TOT), -1.0, np.float32)
    srcrel_pad[core_s, pos] = srel_s.astype(np.int16)
    dstrel_pad[core_s, pos] = drel_s.astype(np.float32)

    # schedule: chunk j -> (t, b); bank stream position q
    tb_of_chunk = np.repeat(np.arange(TILES * NBANK), K.reshape(-1))
    t_of_chunk = tb_of_chunk // NBANK
    b_of_chunk = tb_of_chunk % NBANK
    q_of_chunk = np.zeros(TOTCH, np.int64)
    Cb = np.zeros(NBANK, np.int64)
    for j in range(TOTCH):
        bb = b_of_chunk[j]
        q_of_chunk[j] = Cb[bb]
        Cb[bb] += 1

    # per-bank idx streams, wrapped int16 layout [128, C_b * 8]
    gidx = []
    chunks_src = srcrel_pad.reshape(NCORES, TOTCH, P)
    for bb in range(NBANK):
        sel = chunks_src[:, b_of_chunk == bb, :].reshape(NCORES, -1)
        w = sel.reshape(NCORES, -1, 16).transpose(0, 2, 1)
        gidx.append(np.tile(w, (1, 8, 1)).astype(np.int16))   # [8, 128, C_b*8]

    dstrel_in = dstrel_pad.reshape(NCORES, TOTCH, P).transpose(0, 2, 1)
    dstrel_in = dstrel_in.astype(ml_dtypes.bfloat16)          # [8, 128, TOTCH]

    # padded per-core x (feature-major, bf16) and dinv, permuted
    x = np.asarray(x, dtype=np.float32)
    x_pad = np.zeros((NCORES, NPAD, D), np.float32)
    x_pad[core_of, sidx_of] = x
    xcT = np.ascontiguousarray(x_pad.transpose(0, 2, 1)).astype(
        ml_dtypes.bfloat16
    )  # [8, 128, 12544]

    dinv_pad = np.zeros((NCORES, NPAD), np.float32)
    dinv_pad[core_of, sidx_of] = dinv
    dinv_in = np.ascontiguousarray(
        dinv_pad.reshape(NCORES, TILES, P).transpose(0, 2, 1)
    )  # [8, 128, TILES]

    sched = dict(
        K=K, TOTCH=TOTCH, t_of_chunk=t_of_chunk, b_of_chunk=b_of_chunk,
        q_of_chunk=q_of_chunk, Cb=Cb, core_of=core_of, sidx_of=sidx_of,
    )
    return sched, xcT, dinv_in, dstrel_in, gidx


def _build(sched, Cb, triv):
    from concourse import bass, bacc, mybir, tile
    from concourse.masks import make_identity

    f32 = mybir.dt.float32
    bf16 = mybir.dt.bfloat16
    i16 = mybir.dt.int16

    TOTCH = sched["TOTCH"]
    t_of = sched["t_of_chunk"]
    b_of = sched["b_of_chunk"]
    q_of = sched["q_of_chunk"]

    # first/last chunk flags per tile
    is_first = np.zeros(TOTCH, bool)
    is_last = np.zeros(TOTCH, bool)
    prev_t = -1
    for j in range(TOTCH):
        if t_of[j] != prev_t:
            is_first[j] = True
            if j > 0:
                is_last[j - 1] = True
            prev_t = t_of[j]
    is_last[TOTCH - 1] = True

    nc = bacc.Bacc("TRN2", debug=False, num_devices=NCORES, num_swdge_queues=4,
                   dynamic_dma_scratch_size=DMA_SCRATCH)

    xcT_d = nc.dram_tensor("xcT", [P, NPAD], bf16, kind="ExternalInput")
    dinv_d = nc.dram_tensor("dinv", [P, TILES], f32, kind="ExternalInput")
    dstrel_d = nc.dram_tensor("dstrel", [P, TOTCH], bf16, kind="ExternalInput")
    gidx_d = [
        nc.dram_tensor(f"gidx{bb}", [P, int(Cb[bb]) * 8], i16, kind="ExternalInput")
        for bb in range(NBANK)
    ]
    w_d = [nc.dram_tensor(f"w{l}", [P, D], bf16, kind="ExternalInput") for l in range(3)]
    brep_d = [nc.dram_tensor(f"brep{l}", [P, D], f32, kind="ExternalInput") for l in range(3)]
    grep_d = [nc.dram_tensor(f"grep{l}", [P, D], f32, kind="ExternalInput") for l in range(3)]
    btrep_d = [nc.dram_tensor(f"btrep{l}", [P, D], f32, kind="ExternalInput") for l in range(3)]
    iota_d = nc.dram_tensor("iota", [P, P], bf16, kind="ExternalInput")
    out_d = nc.dram_tensor("out", [NPAD, D], f32, kind="ExternalOutput")

    with tile.TileContext(nc) as tc:
        with (
            tc.tile_pool(name="singles", bufs=1) as singles,
            tc.tile_pool(name="gpool", bufs=GBUFS) as gpool,
            tc.tile_pool(name="spool", bufs=4) as spool,
            tc.tile_pool(name="ln", bufs=3) as lnp,
            tc.tile_pool(name="psacc", bufs=4, space="PSUM") as psacc,
            tc.tile_pool(name="psmm", bufs=2, space="PSUM") as psmm,
            tc.tile_pool(name="pstp", bufs=2, space="PSUM") as pstp,
            tc.tile_pool(name="dram", bufs=1, space="DRAM") as dram,
        ):
            # ---- persistent SBUF state ----
            xcT = singles.tile([P, NPAD], bf16)
            for q in range(NBANK):
                nc.sync.dma_start(
                    out=xcT[:, QB[q] * P:QB[q + 1] * P],
                    in_=xcT_d[:, QB[q] * P:QB[q + 1] * P],
                )
            dinv_t = singles.tile([P, TILES], f32)
            nc.sync.dma_start(out=dinv_t[:], in_=dinv_d[:])
            dstrel_t = singles.tile([P, TOTCH], bf16)
            nc.sync.dma_start(out=dstrel_t[:], in_=dstrel_d[:])
            hhat_sb = singles.tile([P, TILES, D], bf16, name="hhat_sb")

            w_t, brep_t, grep_t, btrep_t = [], [], [], []
            for l in range(3):
                wt = singles.tile([P, D], bf16, name=f"w{l}")
                nc.sync.dma_start(out=wt[:], in_=w_d[l][:])
                w_t.append(wt)
                bt = singles.tile([P, D], f32, name=f"brep{l}")
                nc.sync.dma_start(out=bt[:], in_=brep_d[l][:])
                brep_t.append(bt)
                gt_ = singles.tile([P, D], f32, name=f"grep{l}")
                nc.sync.dma_start(out=gt_[:], in_=grep_d[l][:])
                grep_t.append(gt_)
                btt = singles.tile([P, D], f32, name=f"btrep{l}")
                nc.sync.dma_start(out=btt[:], in_=btrep_d[l][:])
                btrep_t.append(btt)
            idx_t = []
            for bb in range(NBANK):
                it0 = singles.tile([P, int(Cb[bb]) * 8], i16, name=f"idxr{bb}")
                nc.sync.dma_start(out=it0[:], in_=gidx_d[bb][:])
                idx_t.append(it0)
            iota_t = singles.tile([P, P], bf16)
            nc.sync.dma_start(out=iota_t[:], in_=iota_d[:])
            ident_f = singles.tile([P, P], f32)
            make_identity(nc, ident_f[:])
            ident_b = singles.tile([P, P], bf16)
            nc.vector.tensor_copy(out=ident_b[:], in_=ident_f[:])
            eps_t = singles.tile([P, 1], f32)
            nc.vector.memset(eps_t[:], EPS)

            # per-quarter AllGather staging + gathered banks
            agin_q = [
                dram.tile([QROWS[q], D], bf16, name=f"agin{q}")
                for q in range(NBANK)
            ]
            hfull_d = [
                [
                    dram.tile([BANKQ[q], D], bf16, addr_space="Shared",
                              name=f"hfull{l}_{q}")
                    for q in range(NBANK)
                ]
                for l in range(NLAYERS)
            ]
            q_of_slot = np.zeros(TILES, np.int64)
            for q in range(NBANK):
                q_of_slot[QB[q]:QB[q + 1]] = q

            def phase_a(l, t):
                """h = x@W for tile t of layer l; scaled bf16 row into hhat_sb.
                Flushes to the quarter staging buffer and fires that quarter's
                AllGather as soon as its last tile is done (so 3 of 4 AGs hide
                under the previous layer's edge phase)."""
                hps = psmm.tile([P, D], f32, space="PSUM", tag="hps")
                nc.tensor.matmul(
                    out=hps[:],
                    lhsT=xcT[:, t * P:(t + 1) * P],
                    rhs=w_t[l][:],
                    start=True,
                    stop=True,
                )
                nc.vector.scalar_tensor_tensor(
                    out=hhat_sb[:, t, :], in0=hps[:],
                    scalar=dinv_t[:, t:t + 1],
                    in1=brep_t[l][:],
                    op0=mybir.AluOpType.mult, op1=mybir.AluOpType.bypass,
                )
                q = int(q_of_slot[t])
                tq = t - QB[q]
                if tq % 8 == 7 or t == QB[q + 1] - 1:
                    t0 = QB[q] + (tq // 8) * 8
                    nb_ = t - t0 + 1
                    nc.sync.dma_start(
                        out=agin_q[q][(t0 - QB[q]) * P:(t0 - QB[q] + nb_) * P, :]
                        .rearrange("(c p) d -> p c d", p=P),
                        in_=hhat_sb[:, t0:t0 + nb_, :],
                    )
                if t == QB[q + 1] - 1:
                    nc.gpsimd.collective_compute(
                        "AllGather",
                        mybir.AluOpType.bypass,
                        replica_groups=[list(range(NCORES))],
                        ins=[agin_q[q].opt()],
                        outs=[hfull_d[l][q].opt()],
                    )

            # ---- layer 0 phase A + quarter AllGathers (prologue) ----
            for t in range(TILES):
                phase_a(0, t)

            def post_tile(l, t, acc):
                """conv scale + LN (+ReLU, transpose, next-layer phase A)."""
                conv = lnp.tile([P, D], f32, tag="conv")
                nc.vector.scalar_tensor_tensor(
                    out=conv[:], in0=acc[:],
                    scalar=dinv_t[:, t:t + 1],
                    in1=brep_t[l][:],
                    op0=mybir.AluOpType.mult,
                    op1=mybir.AluOpType.bypass if triv["b"][l] else mybir.AluOpType.add,
                )
                stats = lnp.tile([P, 6], f32, tag="stats")
                nc.vector.bn_stats(out=stats[:], in_=conv[:])
                mv = lnp.tile([P, 2], f32, tag="mv")
                nc.vector.bn_aggr(out=mv[:], in_=stats[:])
                rstd = lnp.tile([P, 1], f32, tag="rstd")
                nc.scalar.activation(
                    out=rstd[:], in_=mv[:, 1:2],
                    func=mybir.ActivationFunctionType.Sqrt,
                    bias=eps_t[:],
                )
                nc.vector.reciprocal(out=rstd[:], in_=rstd[:])
                y = lnp.tile([P, D], f32, tag="y")
                nc.vector.scalar_tensor_tensor(
                    out=y[:], in0=conv[:], scalar=mv[:, 0:1],
                    in1=rstd[:].to_broadcast([P, D]),
                    op0=mybir.AluOpType.subtract,
                    op1=mybir.AluOpType.mult,
                )
                if not triv["g"][l]:
                    nc.vector.tensor_mul(out=y[:], in0=y[:], in1=grep_t[l][:])
                if not triv["bt"][l]:
                    nc.vector.tensor_add(out=y[:], in0=y[:], in1=btrep_t[l][:])
                if l == NLAYERS - 1:
                    nc.sync.dma_start(out=out_d[t * P:(t + 1) * P, :], in_=y[:])
                    return
                y_bf = lnp.tile([P, D], bf16, tag="ybf")
                nc.scalar.activation(
                    out=y_bf[:], in_=y[:],
                    func=mybir.ActivationFunctionType.Relu,
                )
                tp = pstp.tile([P, P], bf16, space="PSUM", tag="tp")
                nc.tensor.transpose(out=tp[:], in_=y_bf[:], identity=ident_b[:])
                nc.scalar.copy(out=xcT[:, t * P:(t + 1) * P], in_=tp[:])
                phase_a(l + 1, t)

            for l in range(NLAYERS):
                gtiles = {}
                gq = 0
                stile = None
                acc = None
                pending = None  # emit-shifted post-tile work
                since_first = 0
                for j in range(TOTCH):
                    t, bb, q = int(t_of[j]), int(b_of[j]), int(q_of[j])
                    grp, slot = divmod(q, GATHER_GROUP)
                    gk = (bb, grp)
                    if gk not in gtiles:
                        ng = min(GATHER_GROUP, int(Cb[bb]) - grp * GATHER_GROUP)
                        gt = gpool.tile([P, GATHER_GROUP, P], bf16, tag="gbuf",
                                        name=f"g{l}_{bb}_{grp}")
                        nc.gpsimd.dma_gather(
                            out_ap=gt[:, :ng, :],
                            in_ap=hfull_d[l][bb][:],
                            idxs_ap=idx_t[bb][:, grp * GATHER_GROUP * 8:
                                              (grp * GATHER_GROUP + ng) * 8],
                            num_idxs=ng * P,
                            num_idxs_reg=ng * P,
                            elem_size=P,
                            single_packet=SINGLE_PACKET,
                            queue_num=gq % 4,
                        )
                        gq += 1
                        gtiles[gk] = gt
                    if j % S_BATCH == 0:
                        nb = min(S_BATCH, TOTCH - j)
                        stile = spool.tile([P, S_BATCH, P], bf16, tag="s",
                                           name=f"s{l}_{j}")
                        nc.vector.tensor_tensor(
                            out=stile[:, :nb, :],
                            in0=iota_t[:, None, :].to_broadcast([P, nb, P]),
                            in1=dstrel_t[:, j:j + nb].to_broadcast([P, nb, P]),
                            op=mybir.AluOpType.is_equal,
                        )
                    if is_first[j]:
                        since_first = 0
                        acc = psacc.tile([P, D], f32, space="PSUM", tag="acc",
                                         name=f"acc{l}_{t}")
                        # self-loop message: acc <- I @ hhat_tile
                        nc.tensor.matmul(
                            out=acc[:],
                            lhsT=ident_b[:],
                            rhs=hhat_sb[:, t, :],
                            start=True,
                            stop=False,
                        )
                    nc.tensor.matmul(
                        out=acc[:],
                        lhsT=stile[:, j % S_BATCH, :],
                        rhs=gtiles[gk][:, slot, :],
                        start=False,
                        stop=bool(is_last[j]),
                    )
                    since_first += 1
                    # previous tile's LN/transpose/phase-A, emitted mid-tile so
                    # the PE transpose never waits on the DVE LN chain
                    if pending is not None and since_first == 8:
                        post_tile(*pending)
                        pending = None
                    if is_last[j]:
                        if pending is not None:  # short tile fallback
                            post_tile(*pending)
                        pending = (l, t, acc)
                if pending is not None:
                    post_tile(*pending)

    nc.compile()
    return nc


def _ensure_ntff_hook():
    """The agent image's antenv lacks axon_hooks; synthesize it and register
    the ctypes-based NTFF profile hook so trace=True works."""
    import types

    try:
        from antenv.axon_hooks import get_axon_ntff_profile_hook  # noqa: F401
        return
    except ImportError:
        pass
    import antenv

    mod = types.ModuleType("antenv.axon_hooks")
    mod._hook = None

    def set_axon_ntff_profile_hook(h):
        mod._hook = h

    def get_axon_ntff_profile_hook():
        return mod._hook

    mod.set_axon_ntff_profile_hook = set_axon_ntff_profile_hook
    mod.get_axon_ntff_profile_hook = get_axon_ntff_profile_hook
    sys.modules["antenv.axon_hooks"] = mod
    antenv.axon_hooks = mod
    try:
        from trn_agent_boot.trn_boot import _ntff_profile_via_ctypes

        mod._hook = _ntff_profile_via_ctypes("/opt/axon/libaxon_pjrt.so")
    except Exception as e:  # degrade to no tracing
        print("ntff hook setup failed:", e)


def kernel(**inputs) -> np.ndarray:
    x = np.asarray(inputs["x"], np.float32)
    edge_index = np.asarray(inputs["edge_index"])
    Ws = [np.asarray(inputs[f"W{l}"], np.float32) for l in range(3)]
    bs = [np.asarray(inputs[f"b{l}"], np.float32) for l in range(3)]
    gs = [np.asarray(inputs[f"g{l}"], np.float32) for l in range(3)]
    bts = [np.asarray(inputs[f"bt{l}"], np.float32) for l in range(3)]

    triv = dict(
        b=[bool(np.all(b == 0)) for b in bs],
        g=[bool(np.all(g == 1)) for g in gs],
        bt=[bool(np.all(bt == 0)) for bt in bts],
    )

    sched, xcT, dinv_in, dstrel_in, gidx = _preprocess(x, edge_index)
    nc = _build(sched, sched["Cb"], triv)

    iota = np.broadcast_to(
        np.arange(P, dtype=np.float32), (P, P)
    ).astype(ml_dtypes.bfloat16)

    in_maps = []
    for c in range(NCORES):
        m = dict(
            xcT=np.ascontiguousarray(xcT[c]),
            dinv=np.ascontiguousarray(dinv_in[c]),
            dstrel=np.ascontiguousarray(dstrel_in[c]),
            iota=np.ascontiguousarray(iota),
        )
        for bb in range(NBANK):
            m[f"gidx{bb}"] = np.ascontiguousarray(gidx[bb][c])
        for l in range(3):
            m[f"w{l}"] = Ws[l].astype(ml_dtypes.bfloat16)
            m[f"brep{l}"] = np.ascontiguousarray(
                np.broadcast_to(bs[l], (P, D)).astype(np.float32))
            m[f"grep{l}"] = np.ascontiguousarray(
                np.broadcast_to(gs[l], (P, D)).astype(np.float32))
            m[f"btrep{l}"] = np.ascontiguousarray(
                np.broadcast_to(bts[l], (P, D)).astype(np.float32))
        in_maps.append(m)

    from concourse.bass_utils import run_bass_kernel_spmd

    trace = bool(int(os.environ.get("GCN_TRACE", "0")))
    if trace:
        _ensure_ntff_hook()
    res = run_bass_kernel_spmd(
        nc, in_maps, core_ids=list(range(NCORES)), trace=trace
    )
    kernel.last_results = res

    out = np.zeros((N, D), np.float32)
    core_of = sched["core_of"]
    sidx_of = sched["sidx_of"]
    for c in range(NCORES):
        mask = core_of == c
        out[mask] = res.results[c]["out"][sidx_of[mask]]
    return out


# revision 20
# speedup vs baseline: 1.0729x; 1.0729x over previous
"""3-layer GCN encoder (GCNConv + LayerNorm + ReLU) on 8 TRN2 NeuronCores.

Strategy (dst-partitioned graph parallel, v2):
  - Nodes are packed into 128-row tiles by a host-side bin-packing that
    equalizes per-(tile,bank) gather-group sizes across cores (bimodal tile
    in-degree targets), cutting shared-schedule chunk padding from ~32% to
    ~14%. Tiles are striped across the 8 cores.
  - Self-loop messages never enter the gather stream: each dst tile's PSUM
    accumulator is initialized with hhat_tile via an identity matmul
    (conv = dinv_dst * (sum_gathered hhat_src + hhat_dst) + b).
  - Per layer l: each core computes h_c = x_c @ W_l (bf16), scales rows by
    dinv (deg^-1/2), keeps the bf16 result in SBUF (hhat_sb) and AllGathers
    it so every core holds the full scaled table hhat [100352, 128] bf16 in
    DRAM.
  - Edge phase: edges grouped by (dst_tile, src_bank); each 128-edge chunk
    is gathered from hfull via dma_gather (int16 indices, 4 banks of 25088
    rows) and scatter-added into the dst tile's PSUM via a one-hot matmul.
  - Per dst tile: conv = PSUM * dinv_dst + b, LayerNorm (+ReLU), PE
    transpose back into the feature-major bf16 x_cT buffer, and the NEXT
    layer's x@W matmul runs immediately (interleaved into the edge loop,
    emit-shifted by one tile so the LN chain never stalls the PE stream).

kernel(**inputs) takes the FULL inputs and returns the FULL [100000, 128]
float32 output.
"""
import os
import sys

sys.path.insert(0, "/opt/trn_rl_repo")

import numpy as np
import ml_dtypes

N = 100000
D = 128
NCORES = 8
P = 128
TILES = 98           # tiles per core
NPAD = TILES * P     # 12544 padded nodes per core
NG = NCORES * NPAD   # 100352 global padded rows
NBANK = 4
QB = [0, 25, 50, 74, 98]          # tile-quarter boundaries (per core)
QROWS = [3200, 3200, 3072, 3072]  # rows per quarter per core
BANKQ = [8 * r for r in QROWS]    # bank sizes (< 32767, int16-addressable)
EPS = 1e-5
NLO = 60             # low-degree tiles per core (cells ~450 -> 4 chunks)
T_LO = 1720.0

GATHER_GROUP = int(os.environ.get("GCN_G", "32"))   # chunks per dma_gather
SINGLE_PACKET = bool(int(os.environ.get("GCN_SP", "0")))
S_BATCH = int(os.environ.get("GCN_SB", "16"))       # chunks per is_equal
GBUFS = int(os.environ.get("GCN_GBUFS", "10"))
DMA_SCRATCH = int(os.environ.get("GCN_SCRATCH", "16384"))  # desc-ring carveout
NLAYERS = int(os.environ.get("GCN_LAYERS", "3"))


def _pack_tiles(degp):
    """Assign nodes to (core, slot-in-core, pos) equalizing per-tile in-degree
    sums with bimodal targets, so per-(core,tile,bank) gather groups quantize
    tightly to 128-row chunks under the shared (max-over-cores) schedule."""
    import heapq

    ntiles = NCORES * TILES
    nlo = NCORES * NLO
    tot = float(degp.sum())
    t_hi = (tot / NCORES - NLO * T_LO) / (TILES - NLO)
    targets = np.empty(ntiles)
    targets[:nlo] = T_LO
    targets[nlo:] = t_hi

    order = np.argsort(-degp, kind="stable")
    heap = [(-targets[t], t) for t in range(ntiles)]
    heapq.heapify(heap)
    slots_left = np.full(ntiles, P, np.int32)
    cur = np.zeros(ntiles)
    tile_of = np.empty(N, np.int32)
    for v in order:
        while True:
            negdef, t = heapq.heappop(heap)
            if slots_left[t] > 0:
                break
        tile_of[v] = t
        slots_left[t] -= 1
        cur[t] += degp[v]
        if slots_left[t] > 0:
            heapq.heappush(heap, (-(targets[t] - cur[t]), t))

    gt = np.arange(ntiles)
    core_of_tile = np.where(gt < nlo, gt % NCORES, (gt - nlo) % NCORES)
    slot_of_tile = np.where(gt < nlo, gt // NCORES, NLO + (gt - nlo) // NCORES)

    pos_of = np.empty(N, np.int64)
    cnt_in_tile = np.zeros(ntiles, np.int64)
    for v in np.argsort(tile_of, kind="stable"):
        t = tile_of[v]
        pos_of[v] = cnt_in_tile[t]
        cnt_in_tile[t] += 1

    core_of = core_of_tile[tile_of].astype(np.int64)
    slot_of = slot_of_tile[tile_of].astype(np.int64)
    return core_of, slot_of, pos_of


def _preprocess(x, edge_index):
    """Host-side graph preprocessing. Returns per-core input arrays and the
    shared chunk schedule. Self-loops are excluded from the gather stream."""
    ei = np.asarray(edge_index)
    src = np.asarray(ei[0], np.int64)   # non-self edges only
    dst = np.asarray(ei[1], np.int64)
    M = src.shape[0]

    deg = (np.bincount(dst, minlength=N) + 1).astype(np.float32)  # incl self
    dinv = 1.0 / np.sqrt(deg)

    degp = np.bincount(dst, minlength=N).astype(np.int64)
    core_of, slot_of, pos_of = _pack_tiles(degp)
    sidx_of = slot_of * P + pos_of          # row within the core's slice
    # bank = tile-quarter of the source node; row within bank is
    # core-major over that quarter's rows (matching the quarter AllGather)
    q_of_slot = np.zeros(TILES, np.int64)
    for q in range(NBANK):
        q_of_slot[QB[q]:QB[q + 1]] = q
    qb_rows = np.array([QB[q] * P for q in range(NBANK)], np.int64)
    qrows = np.array(QROWS, np.int64)

    node_q = q_of_slot[slot_of]
    node_srel = core_of * qrows[node_q] + (sidx_of - qb_rows[node_q])

    core = core_of[dst]
    t = slot_of[dst]
    drel = pos_of[dst]
    b = node_q[src]
    srel = node_srel[src]

    key = (core * TILES + t) * NBANK + b
    order = np.argsort(key, kind="stable")
    key_s = key[order]
    core_s = core[order]
    srel_s = srel[order]
    drel_s = drel[order]

    cnt = np.bincount(key, minlength=NCORES * TILES * NBANK).reshape(
        NCORES, TILES, NBANK
    )
    K = np.ceil(cnt.max(axis=0) / P).astype(np.int64)  # [TILES, NBANK] shared
    Ltb = (K * P).reshape(-1)                          # padded group lengths
    off2 = np.concatenate([[0], np.cumsum(Ltb)[:-1]])  # group offsets (flat t,b)
    TOT = int(Ltb.sum())                               # padded edges per core
    TOTCH = TOT // P

    # rank of each edge within its (core, t, b) group
    first = np.searchsorted(key_s, key_s, side="left")
    rank = np.arange(M) - first
    pos = off2[(key_s % (TILES * NBANK))] + rank

    srcrel_pad = np.zeros((NCORES, TOT), np.int16)
    dstrel_pad = np.full((NCORES, TOT), -1.0, np.float32)
    srcrel_pad[core_s, pos] = srel_s.astype(np.int16)
    dstrel_pad[core_s, pos] = drel_s.astype(np.float32)

    # schedule: chunk j -> (t, b); bank stream position q
    tb_of_chunk = np.repeat(np.arange(TILES * NBANK), K.reshape(-1))
    t_of_chunk = tb_of_chunk // NBANK
    b_of_chunk = tb_of_chunk % NBANK
    q_of_chunk = np.zeros(TOTCH, np.int64)
    Cb = np.zeros(NBANK, np.int64)
    for j in range(TOTCH):
        bb = b_of_chunk[j]
        q_of_chunk[j] = Cb[bb]
        Cb[bb] += 1

    # per-bank idx streams, wrapped int16 layout [128, C_b * 8]
    gidx = []
    chunks_src = srcrel_pad.reshape(NCORES, TOTCH, P)
    for bb in range(NBANK):
        sel = chunks_src[:, b_of_chunk == bb, :].reshape(NCORES, -1)
        w = sel.reshape(NCORES, -1, 16).transpose(0, 2, 1)
        gidx.append(np.tile(w, (1, 8, 1)).astype(np.int16))   # [8, 128, C_b*8]

    dstrel_in = dstrel_pad.reshape(NCORES, TOTCH, P).transpose(0, 2, 1)
    dstrel_in = dstrel_in.astype(ml_dtypes.bfloat16)          # [8, 128, TOTCH]

    # padded per-core x (feature-major, bf16) and dinv, permuted
    x = np.asarray(x, dtype=np.float32)
    x_pad = np.zeros((NCORES, NPAD, D), np.float32)
    x_pad[core_of, sidx_of] = x
    xcT = np.ascontiguousarray(x_pad.transpose(0, 2, 1)).astype(
        ml_dtypes.bfloat16
    )  # [8, 128, 12544]

    dinv_pad = np.zeros((NCORES, NPAD), np.float32)
    dinv_pad[core_of, sidx_of] = dinv
    dinv_in = np.ascontiguousarray(
        dinv_pad.reshape(NCORES, TILES, P).transpose(0, 2, 1)
    )  # [8, 128, TILES]

    sched = dict(
        K=K, TOTCH=TOTCH, t_of_chunk=t_of_chunk, b_of_chunk=b_of_chunk,
        q_of_chunk=q_of_chunk, Cb=Cb, core_of=core_of, sidx_of=sidx_of,
    )
    return sched, xcT, dinv_in, dstrel_in, gidx


def _build(sched, Cb, triv):
    from concourse import bass, bacc, mybir, tile
    from concourse.masks import make_identity

    f32 = mybir.dt.float32
    bf16 = mybir.dt.bfloat16
    i16 = mybir.dt.int16

    TOTCH = sched["TOTCH"]
    t_of = sched["t_of_chunk"]
    b_of = sched["b_of_chunk"]
    q_of = sched["q_of_chunk"]

    # first/last chunk flags per tile
    is_first = np.zeros(TOTCH, bool)
    is_last = np.zeros(TOTCH, bool)
    prev_t = -1
    for j in range(TOTCH):
        if t_of[j] != prev_t:
            is_first[j] = True
            if j > 0:
                is_last[j - 1] = True
            prev_t = t_of[j]
    is_last[TOTCH - 1] = True

    nc = bacc.Bacc("TRN2", debug=False, num_devices=NCORES, num_swdge_queues=4,
                   dynamic_dma_scratch_size=DMA_SCRATCH)

    xcT_d = nc.dram_tensor("xcT", [P, NPAD], bf16, kind="ExternalInput")
    dinv_d = nc.dram_tensor("dinv", [P, TILES], f32, kind="ExternalInput")
    dstrel_d = nc.dram_tensor("dstrel", [P, TOTCH], bf16, kind="ExternalInput")
    gidx_d = [
        nc.dram_tensor(f"gidx{bb}", [P, int(Cb[bb]) * 8], i16, kind="ExternalInput")
        for bb in range(NBANK)
    ]
    w_d = [nc.dram_tensor(f"w{l}", [P, D], bf16, kind="ExternalInput") for l in range(3)]
    brep_d = [nc.dram_tensor(f"brep{l}", [P, D], f32, kind="ExternalInput") for l in range(3)]
    grep_d = [nc.dram_tensor(f"grep{l}", [P, D], f32, kind="ExternalInput") for l in range(3)]
    btrep_d = [nc.dram_tensor(f"btrep{l}", [P, D], f32, kind="ExternalInput") for l in range(3)]
    iota_d = nc.dram_tensor("iota", [P, P], bf16, kind="ExternalInput")
    out_d = nc.dram_tensor("out", [NPAD, D], f32, kind="ExternalOutput")

    with tile.TileContext(nc) as tc:
        with (
            tc.tile_pool(name="singles", bufs=1) as singles,
            tc.tile_pool(name="gpool", bufs=GBUFS) as gpool,
            tc.tile_pool(name="spool", bufs=4) as spool,
            tc.tile_pool(name="ln", bufs=3) as lnp,
            tc.tile_pool(name="psacc", bufs=4, space="PSUM") as psacc,
            tc.tile_pool(name="psmm", bufs=2, space="PSUM") as psmm,
            tc.tile_pool(name="pstp", bufs=2, space="PSUM") as pstp,
            tc.tile_pool(name="dram", bufs=1, space="DRAM") as dram,
        ):
            # ---- persistent SBUF state ----
            xcT = singles.tile([P, NPAD], bf16)
            for q in range(NBANK):
                nc.sync.dma_start(
                    out=xcT[:, QB[q] * P:QB[q + 1] * P],
                    in_=xcT_d[:, QB[q] * P:QB[q + 1] * P],
                )
            dinv_t = singles.tile([P, TILES], f32)
            nc.sync.dma_start(out=dinv_t[:], in_=dinv_d[:])
            dstrel_t = singles.tile([P, TOTCH], bf16)
            nc.sync.dma_start(out=dstrel_t[:], in_=dstrel_d[:])
            hhat_sb = singles.tile([P, TILES, D], bf16, name="hhat_sb")

            w_t, brep_t, grep_t, btrep_t = [], [], [], []
            for l in range(3):
                wt = singles.tile([P, D], bf16, name=f"w{l}")
                nc.sync.dma_start(out=wt[:], in_=w_d[l][:])
                w_t.append(wt)
                bt = singles.tile([P, D], f32, name=f"brep{l}")
                nc.sync.dma_start(out=bt[:], in_=brep_d[l][:])
                brep_t.append(bt)
                gt_ = singles.tile([P, D], f32, name=f"grep{l}")
                nc.sync.dma_start(out=gt_[:], in_=grep_d[l][:])
                grep_t.append(gt_)
                btt = singles.tile([P, D], f32, name=f"btrep{l}")
                nc.sync.dma_start(out=btt[:], in_=btrep_d[l][:])
                btrep_t.append(btt)
            idx_t = []
            for bb in range(NBANK):
                it0 = singles.tile([P, int(Cb[bb]) * 8], i16, name=f"idxr{bb}")
                nc.sync.dma_start(out=it0[:], in_=gidx_d[bb][:])
                idx_t.append(it0)
            iota_t = singles.tile([P, P], bf16)
            nc.sync.dma_start(out=iota_t[:], in_=iota_d[:])
            ident_f = singles.tile([P, P], f32)
            make_identity(nc, ident_f[:])
            ident_b = singles.tile([P, P], bf16)
            nc.vector.tensor_copy(out=ident_b[:], in_=ident_f[:])
            eps_t = singles.tile([P, 1], f32)
            nc.vector.memset(eps_t[:], EPS)

            # per-quarter AllGather staging + gathered banks
            agin_q = [
                dram.tile([QROWS[q], D], bf16, name=f"agin{q}")
                for q in range(NBANK)
            ]
            hfull_d = [
                [
                    dram.tile([BANKQ[q], D], bf16, addr_space="Shared",
                              name=f"hfull{l}_{q}")
                    for q in range(NBANK)
                ]
                for l in range(NLAYERS)
            ]
            q_of_slot = np.zeros(TILES, np.int64)
            for q in range(NBANK):
                q_of_slot[QB[q]:QB[q + 1]] = q

            def phase_a(l, t):
                """h = x@W for tile t of layer l; scaled bf16 row into hhat_sb.
                Flushes to the quarter staging buffer and fires that quarter's
                AllGather as soon as its last tile is done (so 3 of 4 AGs hide
                under the previous layer's edge phase)."""
                hps = psmm.tile([P, D], f32, space="PSUM", tag="hps")
                nc.tensor.matmul(
                    out=hps[:],
                    lhsT=xcT[:, t * P:(t + 1) * P],
                    rhs=w_t[l][:],
                    start=True,
                    stop=True,
                )
                nc.vector.scalar_tensor_tensor(
                    out=hhat_sb[:, t, :], in0=hps[:],
                    scalar=dinv_t[:, t:t + 1],
                    in1=brep_t[l][:],
                    op0=mybir.AluOpType.mult, op1=mybir.AluOpType.bypass,
                )
                q = int(q_of_slot[t])
                tq = t - QB[q]
                if tq % 8 == 7 or t == QB[q + 1] - 1:
                    t0 = QB[q] + (tq // 8) * 8
                    nb_ = t - t0 + 1
                    nc.sync.dma_start(
                        out=agin_q[q][(t0 - QB[q]) * P:(t0 - QB[q] + nb_) * P, :]
                        .rearrange("(c p) d -> p c d", p=P),
                        in_=hhat_sb[:, t0:t0 + nb_, :],
                    )
                if t == QB[q + 1] - 1:
                    nc.gpsimd.collective_compute(
                        "AllGather",
                        mybir.AluOpType.bypass,
                        replica_groups=[list(range(NCORES))],
                        ins=[agin_q[q].opt()],
                        outs=[hfull_d[l][q].opt()],
                    )

            # ---- layer 0 phase A + quarter AllGathers (prologue) ----
            for t in range(TILES):
                phase_a(0, t)

            def post_tile(l, t, acc):
                """conv scale + LN (+ReLU, transpose, next-layer phase A)."""
                conv = lnp.tile([P, D], f32, tag="conv")
                nc.vector.scalar_tensor_tensor(
                    out=conv[:], in0=acc[:],
                    scalar=dinv_t[:, t:t + 1],
                    in1=brep_t[l][:],
                    op0=mybir.AluOpType.mult,
                    op1=mybir.AluOpType.bypass if triv["b"][l] else mybir.AluOpType.add,
                )
                stats = lnp.tile([P, 6], f32, tag="stats")
                nc.vector.bn_stats(out=stats[:], in_=conv[:])
                mv = lnp.tile([P, 2], f32, tag="mv")
                nc.vector.bn_aggr(out=mv[:], in_=stats[:])
                rstd = lnp.tile([P, 1], f32, tag="rstd")
                nc.scalar.activation(
                    out=rstd[:], in_=mv[:, 1:2],
                    func=mybir.ActivationFunctionType.Sqrt,
                    bias=eps_t[:],
                )
                nc.vector.reciprocal(out=rstd[:], in_=rstd[:])
                y = lnp.tile([P, D], f32, tag="y")
                nc.vector.scalar_tensor_tensor(
                    out=y[:], in0=conv[:], scalar=mv[:, 0:1],
                    in1=rstd[:].to_broadcast([P, D]),
                    op0=mybir.AluOpType.subtract,
                    op1=mybir.AluOpType.mult,
                )
                if not triv["g"][l]:
                    nc.vector.tensor_mul(out=y[:], in0=y[:], in1=grep_t[l][:])
                if not triv["bt"][l]:
                    nc.vector.tensor_add(out=y[:], in0=y[:], in1=btrep_t[l][:])
                if l == NLAYERS - 1:
                    nc.sync.dma_start(out=out_d[t * P:(t + 1) * P, :], in_=y[:])
                    return
                y_bf = lnp.tile([P, D], bf16, tag="ybf")
                nc.scalar.activation(
                    out=y_bf[:], in_=y[:],
                    func=mybir.ActivationFunctionType.Relu,
                )
                tp = pstp.tile([P, P], bf16, space="PSUM", tag="tp")
                nc.tensor.transpose(out=tp[:], in_=y_bf[:], identity=ident_b[:])
                nc.scalar.copy(out=xcT[:, t * P:(t + 1) * P], in_=tp[:])
                phase_a(l + 1, t)

            def issue_gather(l, gtiles, gk, gq):
                bb, grp = gk
                ng = min(GATHER_GROUP, int(Cb[bb]) - grp * GATHER_GROUP)
                gt = gpool.tile([P, GATHER_GROUP, P], bf16, tag="gbuf",
                                name=f"g{l}_{bb}_{grp}")
                nc.gpsimd.dma_gather(
                    out_ap=gt[:, :ng, :],
                    in_ap=hfull_d[l][bb][:],
                    idxs_ap=idx_t[bb][:, grp * GATHER_GROUP * 8:
                                      (grp * GATHER_GROUP + ng) * 8],
                    num_idxs=ng * P,
                    num_idxs_reg=ng * P,
                    elem_size=P,
                    single_packet=SINGLE_PACKET,
                    queue_num=gq % 4,
                )
                gtiles[gk] = gt

            for l in range(NLAYERS):
                gtiles = {}
                gq = 0
                stile = None
                acc = None
                pending = None  # emit-shifted post-tile work
                since_first = 0
                # pre-issue early groups of banks 0-2 so desc-gen for them
                # runs while the last quarter's AllGather is still in flight
                for grp in range(3):
                    for bb in range(3):
                        if grp * GATHER_GROUP < int(Cb[bb]):
                            issue_gather(l, gtiles, (bb, grp), gq)
                            gq += 1
                for j in range(TOTCH):
                    t, bb, q = int(t_of[j]), int(b_of[j]), int(q_of[j])
                    grp, slot = divmod(q, GATHER_GROUP)
                    gk = (bb, grp)
                    if gk not in gtiles:
                        issue_gather(l, gtiles, gk, gq)
                        gq += 1
                    if j % S_BATCH == 0:
                        nb = min(S_BATCH, TOTCH - j)
                        stile = spool.tile([P, S_BATCH, P], bf16, tag="s",
                                           name=f"s{l}_{j}")
                        nc.vector.tensor_tensor(
                            out=stile[:, :nb, :],
                            in0=iota_t[:, None, :].to_broadcast([P, nb, P]),
                            in1=dstrel_t[:, j:j + nb].to_broadcast([P, nb, P]),
                            op=mybir.AluOpType.is_equal,
                        )
                    if is_first[j]:
                        since_first = 0
                        acc = psacc.tile([P, D], f32, space="PSUM", tag="acc",
                                         name=f"acc{l}_{t}")
                        # self-loop message: acc <- I @ hhat_tile
                        nc.tensor.matmul(
                            out=acc[:],
                            lhsT=ident_b[:],
                            rhs=hhat_sb[:, t, :],
                            start=True,
                            stop=False,
                        )
                    nc.tensor.matmul(
                        out=acc[:],
                        lhsT=stile[:, j % S_BATCH, :],
                        rhs=gtiles[gk][:, slot, :],
                        start=False,
                        stop=bool(is_last[j]),
                    )
                    since_first += 1
                    # previous tile's LN/transpose/phase-A, emitted mid-tile so
                    # the PE transpose never waits on the DVE LN chain
                    if pending is not None and since_first == 8:
                        post_tile(*pending)
                        pending = None
                    if is_last[j]:
                        if pending is not None:  # short tile fallback
                            post_tile(*pending)
                        pending = (l, t, acc)
                if pending is not None:
                    post_tile(*pending)

    nc.compile()
    return nc


def _ensure_ntff_hook():
    """The agent image's antenv lacks axon_hooks; synthesize it and register
    the ctypes-based NTFF profile hook so trace=True works."""
    import types

    try:
        from antenv.axon_hooks import get_axon_ntff_profile_hook  # noqa: F401
        return
    except ImportError:
        pass
    import antenv

    mod = types.ModuleType("antenv.axon_hooks")
    mod._hook = None

    def set_axon_ntff_profile_hook(h):
        mod._hook = h

    def get_axon_ntff_profile_hook():
        return mod._hook

    mod.set_axon_ntff_profile_hook = set_axon_ntff_profile_hook
    mod.get_axon_ntff_profile_hook = get_axon_ntff_profile_hook
    sys.modules["antenv.axon_hooks"] = mod
    antenv.axon_hooks = mod
    try:
        from trn_agent_boot.trn_boot import _ntff_profile_via_ctypes

        mod._hook = _ntff_profile_via_ctypes("/opt/axon/libaxon_pjrt.so")
    except Exception as e:  # degrade to no tracing
        print("ntff hook setup failed:", e)


def kernel(**inputs) -> np.ndarray:
    x = np.asarray(inputs["x"], np.float32)
    edge_index = np.asarray(inputs["edge_index"])
    Ws = [np.asarray(inputs[f"W{l}"], np.float32) for l in range(3)]
    bs = [np.asarray(inputs[f"b{l}"], np.float32) for l in range(3)]
    gs = [np.asarray(inputs[f"g{l}"], np.float32) for l in range(3)]
    bts = [np.asarray(inputs[f"bt{l}"], np.float32) for l in range(3)]

    triv = dict(
        b=[bool(np.all(b == 0)) for b in bs],
        g=[bool(np.all(g == 1)) for g in gs],
        bt=[bool(np.all(bt == 0)) for bt in bts],
    )

    sched, xcT, dinv_in, dstrel_in, gidx = _preprocess(x, edge_index)
    nc = _build(sched, sched["Cb"], triv)

    iota = np.broadcast_to(
        np.arange(P, dtype=np.float32), (P, P)
    ).astype(ml_dtypes.bfloat16)

    in_maps = []
    for c in range(NCORES):
        m = dict(
            xcT=np.ascontiguousarray(xcT[c]),
            dinv=np.ascontiguousarray(dinv_in[c]),
            dstrel=np.ascontiguousarray(dstrel_in[c]),
            iota=np.ascontiguousarray(iota),
        )
        for bb in range(NBANK):
            m[f"gidx{bb}"] = np.ascontiguousarray(gidx[bb][c])
        for l in range(3):
            m[f"w{l}"] = Ws[l].astype(ml_dtypes.bfloat16)
            m[f"brep{l}"] = np.ascontiguousarray(
                np.broadcast_to(bs[l], (P, D)).astype(np.float32))
            m[f"grep{l}"] = np.ascontiguousarray(
                np.broadcast_to(gs[l], (P, D)).astype(np.float32))
            m[f"btrep{l}"] = np.ascontiguousarray(
                np.broadcast_to(bts[l], (P, D)).astype(np.float32))
        in_maps.append(m)

    from concourse.bass_utils import run_bass_kernel_spmd

    trace = bool(int(os.environ.get("GCN_TRACE", "0")))
    if trace:
        _ensure_ntff_hook()
    res = run_bass_kernel_spmd(
        nc, in_maps, core_ids=list(range(NCORES)), trace=trace
    )
    kernel.last_results = res

    out = np.zeros((N, D), np.float32)
    core_of = sched["core_of"]
    sidx_of = sched["sidx_of"]
    for c in range(NCORES):
        mask = core_of == c
        out[mask] = res.results[c]["out"][sidx_of[mask]]
    return out
